# revision 3
# baseline (speedup 1.0000x reference)
"""Trainium2 kernel for nn_MeshTorchLayer_82059645157414.

The 256 sequential MZI mesh layers are linear in the state, so the whole mesh
(including the gamma phase layer) collapses into one 256x256 complex matrix U
with out[b, :] = U @ x[b, :]. U is composed on host in float64; the device
work per core is one real bf16 matmul out[512, 512] = xT.T[512, 256] @ W[256,
512] (W = interleaved re/im columns of U^T), data-parallel over 8 NeuronCores
(512 batch rows each). Overall rel err 2.9e-3 vs the f32 reference (budget
2e-2): bf16 inputs + bf16 output staging.

Per-core schedule (raw Bass, explicit semaphores), driven by SCHED:
  - inputs stream as 4 packed bf16 DMAs sized so the DMA engines run
    back-to-back from the first byte (~2.3us fixed dispatch latency, then
    1.46us of streaming): SP HWDGE #1 [Wk0|x00|x01], Pool SWDGE [Wk1|x10]
    (its descriptor generation overlaps DMA 1's HWDGE path so the transfer
    chains with no gap), SP #2 [x11|x02], SP #3 [x12|x03|x13];
  - PE: one warmup matmul (anchors the p-state ramp clock), then 8
    accumulating bf16 matmuls (213ns each) ordered k0(t0), k0(t1), k1(t0),
    k1(t1), k0(t2), k1(t2), k0(t3), k1(t3): the two k0-front matmuls fill
    the window before the Pool chunk's semaphore (+900ns DMA sem latency)
    and the first tile completes as early as the input ladder allows, after
    which both PSUM-copy engines stay saturated;
  - as each tile's accumulation closes, DVE (cols 0-250) and Act (250-512)
    copy its PSUM tile to SBUF as bf16 (the host converts back to f32);
  - output rides 2 kv_writeback prep+trigger pairs (tiles 0-2, tile 3): the
    descriptors are generated early on GPSIMD (prepare_only) and trigger_dma
    fires them when the copies land, skipping the per-DMA HWDGE queue + DGE
    dispatch latency; the cost of a writeback is descriptor-count based
    (batch*d_head/16+1), far below byte bandwidth.

Empirical real-TRN2 constraints honored here (the cost model is unaware):
  - the DVE|Act boundary column in the PSUM->SBUF copies must be <= 250
    (>= 252 corrupts the boundary columns), DVE on the low side;
  - semaphore waits for DVE/Act copies must be standalone wait_ge
    instructions (instruction-attached waits are silently dropped by the
    real codegen -> garbage data); attached waits do work on the GPSIMD
    kv_writeback preps and trigger_dma.
"""

import numpy as np
import ml_dtypes

import concourse.bass as bass
import concourse.mybir as mybir
from concourse.bass_utils import run_bass_kernel_spmd

UNITS = 256
LAYERS = 256
BATCH = 4096
NCORES = 8
BC = BATCH // NCORES   # 512 rows/core
P = 128
NT = BC // P           # 4 tiles
WF = 2 * UNITS         # 512 out cols

BF16 = mybir.dt.bfloat16
F32 = mybir.dt.float32

# ---------------------------------------------------------------------------
# Schedule config.
# Items: ('w', k, lo, hi) = W[k*128:(k+1)*128, lo:hi] ;  ('x', k, t) = x chunk
# Queues: 'sp' (SP HWDGE, in order), 'pool' (SWDGE desc-gen), 'act' (Act HWDGE)
# ---------------------------------------------------------------------------
SCHED = dict(
    chunks=[
        ('sp',   [('w', 0, 0, 512), ('x', 0, 0), ('x', 0, 1)]),
        ('pool', [('w', 1, 0, 512), ('x', 1, 0)]),
        ('sp',   [('x', 1, 1), ('x', 0, 2)]),
        ('sp',   [('x', 1, 2), ('x', 0, 3), ('x', 1, 3)]),
    ],
    # matmuls: (k, t, lo, hi); k0 starts the accumulation group, k1 stops it.
    pe_order=[
        (0, 0, 0, 512), (0, 1, 0, 512),
        (1, 0, 0, 512), (1, 1, 0, 512),
        (0, 2, 0, 512), (1, 2, 0, 512),
        (0, 3, 0, 512), (1, 3, 0, 512),
    ],
    # copies: (engine, tile, lo, hi[, wait_n]) in per-engine program order.
    # HARDWARE CONSTRAINTS (measured on real TRN2, cost model unaware):
    #  - the DVE|Act boundary column must be <= 250 (>= 252 corrupts);
    #  - DVE must take the low [0:250] slice and Act the high [250:512] one
    #    (swapped sides crash the runtime);
    #  - engine waits must be standalone wait_ge instructions for DVE/Act
    #    copies (instruction-attached waits crash the runtime); attached
    #    waits are fine on gpsimd kv_writeback preps and trigger_dma.
    copies=[
        ('dve', 0, 0, 250), ('act', 0, 250, 512),
        ('dve', 1, 0, 250), ('act', 1, 250, 512),
        ('dve', 2, 0, 250), ('act', 2, 250, 512),
        ('dve', 3, 0, 250), ('act', 3, 250, 512),
    ],
    # writeback groups (t_lo, t_hi): one kv_writeback prep+trigger per group.
    wb_groups=[(0, 3), (3, 4)],
    warmup=1,
    out_bf16=True,
    attach_prep=True,
    attach_trig=True,
)


def _build_w(theta, phi, gamma, mask):
    theta = np.asarray(theta, np.float64)
    phi = np.asarray(phi, np.float64)
    gamma = np.asarray(gamma, np.float64)
    mask = np.asarray(mask)
    L, M = theta.shape
    N = 2 * M
    m = mask.astype(np.float64)
    th = theta * m + (1 - m) * np.pi
    ph = phi * m + (1 - m) * np.pi
    u = np.exp(1j * th)
    e = np.exp(1j * ph)
    d_top = e * (u - 1) * 0.5
    d_bot = (1 - u) * 0.5
    o_top = 1j * (u + 1) * 0.5
    o_bot = 1j * e * (u + 1) * 0.5
    D = np.stack([d_top, d_bot], axis=-1).reshape(L, N)
    O = np.stack([o_top, o_bot], axis=-1).reshape(L, N)
    odd = (np.arange(L) % 2).astype(bool)
    D[odd] = np.roll(D[odd], 1, axis=1)
    O[odd] = np.roll(O[odd], 1, axis=1)
    base = np.arange(N).reshape(-1, 2)[:, ::-1].reshape(-1)
    oddp = np.concatenate([[0], base[:-2] + 1, [N - 1]])
    U = np.diag(np.exp(1j * gamma)).astype(np.complex128)
    for layer in range(L):
        p = oddp if (layer % 2) else base
        U = D[layer][:, None] * U + O[layer][:, None] * U[p, :]
    W = np.empty((N, 2 * N), np.float32)
    W[:, 0::2] = U.real.T.astype(np.float32)
    W[:, 1::2] = U.imag.T.astype(np.float32)
    return W


def _item_cols(item):
    if item[0] == 'w':
        return item[3] - item[2]
    return P


def _build_bass(sched=SCHED):
    nc = bass.Bass()
    chunks = sched['chunks']
    nchunk = len(chunks)
    chunk_cols = [sum(_item_cols(i) for i in items) for _, items in chunks]

    # item -> (chunk index, col offset in chunk sbuf tensor)
    wloc = []  # (k, lo, hi, chunk, off)
    xloc = {}  # (k, t) -> (chunk, off)
    for ci, (_, items) in enumerate(chunks):
        off = 0
        for it in items:
            if it[0] == 'w':
                wloc.append((it[1], it[2], it[3], ci, off))
            else:
                xloc[(it[1], it[2])] = (ci, off)
            off += _item_cols(it)

    def w_ap(sb, k, lo, hi):
        """SBUF AP for W[k][:, lo:hi]; must be inside one chunk item."""
        for (kk, wlo, whi, ci, off) in wloc:
            if kk == k and wlo <= lo and hi <= whi:
                return sb[ci][:, off + (lo - wlo): off + (hi - wlo)]
        raise KeyError((k, lo, hi))

    def w_chunk_of(k, lo, hi):
        for (kk, wlo, whi, ci, off) in wloc:
            if kk == k and wlo <= lo and hi <= whi:
                return ci
        raise KeyError((k, lo, hi))

    ins = [nc.dram_tensor(f"in{ci}", [P, chunk_cols[ci]], BF16,
                          kind="ExternalInput") for ci in range(nchunk)]
    ODT = BF16 if sched.get("out_bf16") else F32
    out = nc.dram_tensor("out", [NT, P, 1, WF], ODT, kind="ExternalOutput")

    pe_order = sched['pe_order']
    copies = sched['copies']
    wb_groups = sched['wb_groups']
    ngrp = len(wb_groups)

    # per-tile: number of matmuls that finish it (stop matmuls) and copies
    tile_cols_done = {}
    for (k, t, lo, hi) in pe_order:
        tile_cols_done.setdefault(t, set()).add((k, lo, hi))
    # mm sem target per tile = count of k=1 (stop) matmuls for that tile
    mm_target = {t: sum(1 for (k, l, h) in s if k == 1)
                 for t, s in tile_cols_done.items()}
    # normalize copies to (eng, t, lo, hi, wait_n)
    copies = [(c[0], c[1], c[2], c[3],
               c[4] if len(c) > 4 else mm_target[c[1]]) for c in copies]
    cp_of_tile = {}
    for eng, t, lo, hi, n in copies:
        cp_of_tile.setdefault(t, []).append((eng, lo, hi))
    grp_cp_target = [sum(len(cp_of_tile[t]) for t in range(tlo, thi))
                     for (tlo, thi) in wb_groups]

    from contextlib import ExitStack
    with ExitStack() as st:
        sb = [st.enter_context(nc.sbuf_tensor(f"c{ci}", [P, chunk_cols[ci]],
                                              BF16)) for ci in range(nchunk)]
        o_sb = st.enter_context(nc.sbuf_tensor("o_sb", [P, 1, NT, WF], ODT))
        spin_d = sched.get('dve_spin', 0)
        spin_a = sched.get('act_spin', 0)
        spin_sb = (st.enter_context(nc.sbuf_tensor(
            "spin_sb", [P, max(spin_d, spin_a, 1)], F32))
            if (spin_d or spin_a) else None)
        warm_sb = st.enter_context(nc.sbuf_tensor("warm_sb", [P, P], BF16))
        idx_sb = st.enter_context(nc.sbuf_tensor("idx_sb", [P, NT],
                                                 mybir.dt.int32))
        acc4 = st.enter_context(nc.psum_tensor("acc4", [P, NT, WF], F32))
        warm_ps = st.enter_context(nc.psum_tensor("warm_ps", [P, P], F32))
        ch_sems = [st.enter_context(nc.semaphore(name=f'ch{ci}_sem'))
                   for ci in range(nchunk)]
        ws_sem = st.enter_context(nc.semaphore(name='ws_sem'))
        idx_sem = st.enter_context(nc.semaphore(name='idx_sem'))
        prep_sem = st.enter_context(nc.semaphore(name='prep_sem'))
        mm_sems = [st.enter_context(nc.semaphore(name=f'mm{t}_sem'))
                   for t in range(NT)]
        cp_sems = [st.enter_context(nc.semaphore(name=f'cp{g}_sem'))
                   for g in range(ngrp)]
        out_sem = st.enter_context(nc.semaphore(name='out_sem'))
        block = st.enter_context(nc.Block())

        def x_ap(k, t):
            ci, off = xloc[(k, t)]
            return sb[ci][:, off: off + P]

        # ---- DMA queues ----
        sp_chunks = [ci for ci, (q, _) in enumerate(chunks) if q == 'sp']
        pool_chunks = [ci for ci, (q, _) in enumerate(chunks) if q == 'pool']
        act_chunks = [ci for ci, (q, _) in enumerate(chunks) if q == 'act']

        @block.sync
        def _(sync):
            for ci in sp_chunks:
                sync.dma_start(sb[ci][:], ins[ci][:]).then_inc(ch_sems[ci], 16)

        @block.gpsimd
        def _(gpsimd):
            for ci in pool_chunks:
                gpsimd.dma_start(sb[ci][:], ins[ci][:]).then_inc(
                    ch_sems[ci], 16)
            from concourse import library_config
            gpsimd.load_library(library_config.attn)
            for g, (tlo, thi) in enumerate(wb_groups):
                if g == 0 and not sched.get('attach_prep', True):
                    gpsimd.wait_ge(idx_sem, 1)
                wb = gpsimd.kv_writeback(
                    out[tlo:thi], o_sb[:, :, tlo:thi, :],
                    idx_sb[:, tlo:thi],
                    prepare_only=True, sem=out_sem,
                ).then_inc(prep_sem, 1)
                if g == 0 and sched.get('attach_prep', True):
                    wb._wait_ge(idx_sem, 1)
            for g in range(ngrp):
                gpsimd.wait_ge(prep_sem, g + 1)
                if sched.get('attach_trig', True):
                    gpsimd.trigger_dma(1)._wait_ge(
                        cp_sems[g], grp_cp_target[g])
                else:
                    gpsimd.wait_ge(cp_sems[g], grp_cp_target[g])
                    gpsimd.trigger_dma(1)

        @block.vector
        def _(vector):
            vector.memset(idx_sb[:], 0).then_inc(idx_sem, 1)
            vector.memset(warm_sb[:], 0.0).then_inc(ws_sem, 1)
            if spin_d:
                vector.wait_ge(ch_sems[0], 16)
                vector.tensor_copy(spin_sb[:, :spin_d], spin_sb[:, :spin_d])
            for eng, t, lo, hi, wn in copies:
                if eng != 'dve':
                    continue
                g = next(g for g, (tlo, thi) in enumerate(wb_groups)
                         if tlo <= t < thi)
                if not sched.get('attach_cp_dve', False):
                    vector.wait_ge(mm_sems[t], wn)
                cp = vector.tensor_copy(o_sb[:, 0, t, lo:hi],
                                        acc4[:, t, lo:hi]
                                        ).then_inc(cp_sems[g], 1)
                if sched.get('attach_cp_dve', False):
                    cp._wait_ge(mm_sems[t], wn)

        @block.scalar
        def _(scalar):
            for ci in act_chunks:
                # hold behind DVE memset so SP wins HWDGE slot 1
                scalar.wait_ge(idx_sem, 1)
                scalar.dma_start(sb[ci][:], ins[ci][:]).then_inc(
                    ch_sems[ci], 16)
            if spin_a:
                scalar.wait_ge(ch_sems[0], 16)
                scalar.copy(spin_sb[:, :spin_a], spin_sb[:, :spin_a])
            for eng, t, lo, hi, wn in copies:
                if eng != 'act':
                    continue
                g = next(g for g, (tlo, thi) in enumerate(wb_groups)
                         if tlo <= t < thi)
                if not sched.get('attach_cp_act', False):
                    scalar.wait_ge(mm_sems[t], wn)
                cp = scalar.copy(o_sb[:, 0, t, lo:hi], acc4[:, t, lo:hi]
                                 ).then_inc(cp_sems[g], 1)
                if sched.get('attach_cp_dve', False):
                    cp._wait_ge(mm_sems[t], wn)

        @block.tensor
        def _(tensor):
            tensor.wait_ge(ws_sem, 1)
            for i in range(sched['warmup']):
                nc.tensor.matmul(warm_ps[:], warm_sb[:], warm_sb[:],
                                 start=True, stop=True)
            waited = set()
            for (k, t, lo, hi) in pe_order:
                ci_w = w_chunk_of(k, lo, hi)
                ci_x = xloc[(k, t)][0]
                for ci in dict.fromkeys((ci_w, ci_x)):
                    if ci not in waited:
                        tensor.wait_ge(ch_sems[ci], 16)
                        waited.add(ci)
                mm = nc.tensor.matmul(acc4[:, t, lo:hi], x_ap(k, t),
                                      w_ap(sb, k, lo, hi),
                                      start=(k == 0), stop=(k == 1),
                                      skip_group_check=True)
                if k == 1:
                    mm.then_inc(mm_sems[t], 1)

    from concourse.library_overlay import lower_extended_insts
    lower_extended_insts(nc)
    return nc


def _pack_inputs(x, W, sched=SCHED):
    xT = np.ascontiguousarray(
        x.reshape(NCORES, BC, UNITS).transpose(0, 2, 1)
    ).astype(ml_dtypes.bfloat16)              # [8, 256, 512]
    Wb = W.astype(ml_dtypes.bfloat16)         # [256, 512]
    per_core = [[] for _ in range(NCORES)]
    for (_, items) in sched['chunks']:
        for c in range(NCORES):
            cols = []
            for it in items:
                if it[0] == 'w':
                    _, k, lo, hi = it
                    cols.append(Wb[k * P:(k + 1) * P, lo:hi])
                else:
                    _, k, t = it
                    cols.append(xT[c, k * P:(k + 1) * P, t * P:(t + 1) * P])
            per_core[c].append(np.ascontiguousarray(np.concatenate(cols, 1)))
    return per_core


def kernel(x, theta, phi, gamma, mask):
    x = np.ascontiguousarray(np.asarray(x, dtype=np.float32))
    assert x.shape == (BATCH, UNITS)
    W = _build_w(theta, phi, gamma, mask)
    packs = _pack_inputs(x, W)
    nc = _build_bass()
    in_maps = [{f"in{ci}": packs[c][ci] for ci in range(len(SCHED['chunks']))}
               for c in range(NCORES)]
    res = run_bass_kernel_spmd(nc, in_maps, core_ids=list(range(NCORES)))
    outs = [np.asarray(r["out"]).reshape(BC, WF) for r in res.results]
    if SCHED.get("out_bf16"):
        outs = [o.astype(np.float32) for o in outs]
    full = np.ascontiguousarray(np.concatenate(outs, axis=0))
    return full.view(np.complex64)
